# revision 9
# baseline (speedup 1.0000x reference)
"""Trainium2 Bass kernel for nn_DecoderRNN: 64-step 2-layer tanh RNN + per-step FC.

Sharding (8 cores, no collectives):
  - 2-way data parallel over batch (cores 0-3: rows 0:128, cores 4-7: rows 128:256).
    Each group of 4 cores redundantly computes its batch-half's RNN.
  - 4-way tensor parallel over the fc output dim (O=8192 -> 2048 per core).

Numerics: RNN matmuls run in float32r (single pass, full-rate for moving
dim >= 256 rows, hardware-internal reduced-precision fp32). The recurrent
state stays fp32 end-to-end. The FC runs in bf16 (logits are not recurrent;
bf16 product noise ~1e-3 does not accumulate).

Per-core compute, per step t (B=128, H=1024, O_slice=2048):
  - State kept transposed ("g" form, [H, B]): g tiles are the matmul stationary
    operand (lhsT), weights stream as the moving operand at N=512.
  - RNN bias enters PSUM via K=1 matmuls against a ones row. tanh on the
    scalar engine in fp32.
  - Layer outputs ([B, H] in PSUM) are transposed back to g form on the tensor
    engine (fp32r transposes), drained by DVE copies; a bf16 cast of g_h1
    feeds the FC.
  - FC bias is added during the DVE PSUM drain (broadcast bias tile), and FC
    matmul chunks are interleaved into the tanh/transpose latency gaps of the
    NEXT step to keep the PE array busy.
"""
import sys

sys.path.insert(0, "/opt/trn_rl_repo")

from contextlib import ExitStack

import numpy as np
import ml_dtypes

import concourse.bass as bass
import concourse.tile as tile
from concourse import bacc, mybir
from concourse.bass_utils import run_bass_kernel_spmd

H = 1024
O = 8192
L = 2
T = 64
B = 256
N_CORES = 8
BG = B // 2          # batch rows per core (2-way DP)
OS = O // 4          # fc output slice per core (4-way TP)
KT = H // 128        # 8 k-tiles per 1024 contraction
F32 = mybir.dt.float32
F32R = mybir.dt.float32r
BF16 = mybir.dt.bfloat16

_cached = {}

RNN_W_NAMES = ["ih0", "hh0", "ih1", "hh1"]


def _build_program(n_steps: int, skip_fc=False, skip_tr=False, skip_rnn=False):
    nc = bacc.Bacc("TRN2", target_bir_lowering=False, debug=False, num_devices=N_CORES)

    # --- DRAM parameters (per-core shards, host-prepared layouts) ---
    # RNN weights, transposed+tiled on host: [p][k][n] of W.T, fp32
    wd = {}
    for nm in RNN_W_NAMES:
        wd[nm] = nc.declare_dram_parameter(f"w_{nm}", [128, KT, H], F32R,
                                           isOutput=False)
    w_fc = nc.declare_dram_parameter("w_fc", [128, KT, OS], BF16, isOutput=False)
    # initial state, g form: [p][k][b] = state[b, k*128+p], fp32
    gd = {}
    for nm in ("x", "h0", "h1"):
        gd[nm] = nc.declare_dram_parameter(f"g_{nm}", [128, KT, BG], F32R,
                                           isOutput=False)
    # bias bcast tiles (b_ih + b_hh per layer), fc bias bcast, identity
    bd = {}
    for l in range(L):
        bd[f"b{l}"] = nc.declare_dram_parameter(f"b{l}", [128, H], F32,
                                                isOutput=False)
    fcbd = nc.declare_dram_parameter("fcb", [128, OS], BF16, isOutput=False)
    identd = nc.declare_dram_parameter("ident", [128, 128], F32R, isOutput=False)

    out_d = nc.declare_dram_parameter("out", [n_steps, 128, OS], F32, isOutput=True)

    with tile.TileContext(nc) as tc, ExitStack() as ctx:
        wpool = ctx.enter_context(tc.tile_pool(name="w", bufs=1))
        cpool = ctx.enter_context(tc.tile_pool(name="c", bufs=1))
        gp = ctx.enter_context(tc.tile_pool(name="gp", bufs=3))
        gbp = ctx.enter_context(tc.tile_pool(name="gbp", bufs=2))
        hp = ctx.enter_context(tc.tile_pool(name="h", bufs=2))
        logp = ctx.enter_context(tc.tile_pool(name="log", bufs=3))
        rnn_ps = ctx.enter_context(tc.tile_pool(name="rnnps", bufs=2, space="PSUM"))
        tr_ps = ctx.enter_context(tc.tile_pool(name="trps", bufs=2, space="PSUM"))
        fc_ps = ctx.enter_context(tc.tile_pool(name="fcps", bufs=2, space="PSUM"))

        # --- preamble: load weights/constants ---
        w = {}
        for nm, dram in wd.items():
            t_ = wpool.tile([128, KT, H], F32R, tag=f"w{nm}", name=f"w{nm}")
            nc.sync.dma_start(t_[:], dram[:])
            w[nm] = t_
        wfc = wpool.tile([128, KT, OS], BF16, tag="wfc")
        nc.sync.dma_start(wfc[:], w_fc[:])

        bb = {}
        for nm, dram in bd.items():
            t_ = cpool.tile([128, H], F32, tag=nm, name=nm)
            nc.sync.dma_start(t_[:], dram[:])
            bb[nm] = t_
        fcb = cpool.tile([128, OS], BF16, tag="fcb")
        ident = cpool.tile([128, 128], F32R, tag="ident")
        for t_, d_ in [(fcb, fcbd), (ident, identd)]:
            nc.sync.dma_start(t_[:], d_[:])

        # --- initial state ---
        def g_init(nm):
            g = gp.tile([128, KT, BG], F32R, tag="g", name="g")
            nc.sync.dma_start(g[:], gd[nm][:])
            return g

        g_x = g_init("x")
        g_h0 = g_init("h0")
        g_h1 = g_init("h1")
        # bf16 copy of g_h1 for the FC (cast once on DVE)
        g_h1b = gbp.tile([128, KT, BG], BF16, tag="gb", name="gb")
        nc.vector.tensor_copy(g_h1b[:], g_h1[:])

        def rnn_layer(g_in, g_h, w_in, w_h, b):
            """tanh(in @ W_ihT + h @ W_hhT + b) -> h_sbuf [128(B), H] f32r.

            Bias is added in-place on the DVE (PSUM += bias bcast tile), off
            the PE array; tanh on the scalar engine."""
            ps = rnn_ps.tile([128, H], F32, tag="rnnps")
            for nck in range(2):
                nsl = bass.ts(nck, 512)
                for pi, (lhs, rhs) in enumerate(((g_in, w_in), (g_h, w_h))):
                    for k in range(KT):
                        first = pi == 0 and k == 0
                        last = pi == 1 and k == KT - 1
                        nc.tensor.matmul(ps[:, nsl], lhs[:, k, :], rhs[:, k, nsl],
                                         start=first, stop=last)
                nc.vector.tensor_add(ps[:, nsl], ps[:, nsl], b[:, nsl])
            h_sb = hp.tile([128, H], F32R, tag="h")
            nc.scalar.activation(h_sb[:], ps[:], mybir.ActivationFunctionType.Tanh)
            return h_sb

        def to_g(h_sb, want_bf16=False):
            """PE-transpose [B, H] -> g form [H(p), B] (fp32r transposes).

            4 transposed 128x128 tiles per PSUM bank; each bank drained by a
            wide DVE copy. Optionally also emit a bf16 copy for the FC."""
            g = gp.tile([128, KT, BG], F32R, tag="g", name="g")
            gb = None
            if want_bf16:
                gb = gbp.tile([128, KT, BG], BF16, tag="gb", name="gb")
            for grp in range(2):
                pt = tr_ps.tile([128, 512], F32R, tag="trps", name="pt")
                for j in range(4):
                    k = grp * 4 + j
                    nc.tensor.transpose(pt[:, bass.ts(j, 128)],
                                        h_sb[:, bass.ts(k, 128)], ident[:])
                gs = g[:, grp * 4:(grp + 1) * 4, :]
                nc.vector.tensor_copy(gs, pt[:])
                if want_bf16:
                    nc.vector.tensor_copy(gb[:, grp * 4:(grp + 1) * 4, :], pt[:])
            return g, gb

        def emit_fc_chunk(gb, tprev, ci):
            """FC chunk: logits[:, ci*512:(ci+1)*512] for step tprev (bf16)."""
            ps = fc_ps.tile([128, 512], F32, tag="fcps", name="fps")
            fsl = bass.ts(ci, 512)
            for k in range(KT):
                nc.tensor.matmul(ps[:], gb[:, k, :], wfc[:, k, fsl],
                                 start=(k == 0), stop=(k == KT - 1))
            lsb = logp.tile([128, 512], F32, tag="log", name="lsb")
            nc.vector.tensor_add(lsb[:], ps[:], fcb[:, fsl])
            nc.sync.dma_start(out_d[tprev][:, fsl], lsb[:])

        # Software pipeline: FC of step t-1 is interleaved into step t's
        # tanh/transpose gaps. pending = (g_h1_bf16, t_index) awaiting FC.
        pending = (g_h1b, None)  # g from init; no FC for it

        for t in range(n_steps):
            gb_prev, tprev = pending
            if not skip_rnn:
                h0_sb = rnn_layer(g_x, g_h0, w["ih0"], w["hh0"], bb["b0"])
                if not skip_fc and tprev is not None:
                    emit_fc_chunk(gb_prev, tprev, 0)
                if not skip_tr:
                    g_h0, _ = to_g(h0_sb)
                if not skip_fc and tprev is not None:
                    emit_fc_chunk(gb_prev, tprev, 1)
                h1_sb = rnn_layer(g_h0, g_h1, w["ih1"], w["hh1"], bb["b1"])
                if not skip_fc and tprev is not None:
                    emit_fc_chunk(gb_prev, tprev, 2)
                if not skip_tr:
                    g_h1, g_h1b = to_g(h1_sb, want_bf16=True)
                if not skip_fc and tprev is not None:
                    emit_fc_chunk(gb_prev, tprev, 3)
                g_x = g_h1
            pending = (g_h1b, t)

        # drain the last step's FC
        gb_prev, tprev = pending
        if not skip_fc and tprev is not None:
            for ci in range(4):
                emit_fc_chunk(gb_prev, tprev, ci)

    nc.finalize()
    return nc


def _prep_inputs(x, hidden, W_ih, W_hh, b_ih, b_hh, fc_W, fc_b, n_steps):
    """Build the 8 per-core input maps (host-side transposes)."""
    def gform(a):  # [BG, H] f32 -> [128, KT, BG]: out[p, k, b] = a[b, k*128+p]
        return np.ascontiguousarray(
            a.T.reshape(KT, 128, BG).transpose(1, 0, 2)).astype(np.float32)

    def wform(Wmat):  # [H_out, H_in] -> [128, KT, H_out] of W.T (f32)
        return np.ascontiguousarray(
            Wmat.T.reshape(KT, 128, Wmat.shape[0]).transpose(1, 0, 2)).astype(
                np.float32)

    ident = np.eye(128, dtype=np.float32)

    common = {"ident": ident}
    for l, nm_pair in enumerate([("ih0", "hh0"), ("ih1", "hh1")]):
        for nm, Wmat in zip(nm_pair, (W_ih[l], W_hh[l])):
            common[f"w_{nm}"] = wform(Wmat)
        common[f"b{l}"] = np.broadcast_to(
            (b_ih[l] + b_hh[l]).astype(np.float32).reshape(1, H),
            (128, H)).copy()

    in_maps = []
    for c in range(N_CORES):
        bg, j = c // 4, c % 4
        bsl = slice(bg * BG, (bg + 1) * BG)
        osl = slice(j * OS, (j + 1) * OS)
        wfc = np.ascontiguousarray(
            fc_W[osl].T.reshape(KT, 128, OS).transpose(1, 0, 2)).astype(
                ml_dtypes.bfloat16)
        m = dict(common)
        m["w_fc"] = wfc
        m["fcb"] = np.broadcast_to(
            fc_b[osl].astype(ml_dtypes.bfloat16).reshape(1, OS),
            (128, OS)).copy()
        for nm, src in (("x", x[0, bsl]), ("h0", hidden[0, bsl]),
                        ("h1", hidden[1, bsl])):
            m[f"g_{nm}"] = gform(src)
        in_maps.append(m)
    return in_maps


def kernel(x, hidden, embedded, W_ih, W_hh, b_ih, b_hh, fc_W, fc_b,
           _trace=False, _trace_kwargs=None):
    n_steps = embedded.shape[0]
    key = n_steps
    if key not in _cached:
        _cached[key] = _build_program(n_steps)
    nc = _cached[key]

    in_maps = _prep_inputs(np.asarray(x), np.asarray(hidden), np.asarray(W_ih),
                           np.asarray(W_hh), np.asarray(b_ih), np.asarray(b_hh),
                           np.asarray(fc_W), np.asarray(fc_b), n_steps)
    core_ids = list(range(N_CORES))
    res = run_bass_kernel_spmd(nc, in_maps, core_ids, trace=_trace,
                               **(_trace_kwargs or {}))

    out = np.empty((n_steps, 1, B, O), np.float32)
    for c in range(N_CORES):
        bg, j = c // 4, c % 4
        out[:, 0, bg * BG:(bg + 1) * BG, j * OS:(j + 1) * OS] = res.results[c]["out"]
    if _trace:
        kernel.last_results = res
    return out


# revision 10
# speedup vs baseline: 1.0018x; 1.0018x over previous
"""Trainium2 Bass kernel for nn_DecoderRNN: 64-step 2-layer tanh RNN + per-step FC.

Sharding (8 cores, no collectives):
  - 2-way data parallel over batch (cores 0-3: rows 0:128, cores 4-7: rows 128:256).
    Each group of 4 cores redundantly computes its batch-half's RNN.
  - 4-way tensor parallel over the fc output dim (O=8192 -> 2048 per core).

Numerics: RNN matmuls run in float32r (single pass, full-rate for moving
dim >= 256 rows, hardware-internal reduced-precision fp32). The recurrent
state stays fp32 end-to-end. The FC runs in bf16 (logits are not recurrent;
bf16 product noise ~1e-3 does not accumulate).

Per-core compute, per step t (B=128, H=1024, O_slice=2048):
  - State kept transposed ("g" form, [H, B]): g tiles are the matmul stationary
    operand (lhsT), weights stream as the moving operand at N=512.
  - RNN bias enters PSUM via K=1 matmuls against a ones row. tanh on the
    scalar engine in fp32.
  - Layer outputs ([B, H] in PSUM) are transposed back to g form on the tensor
    engine (fp32r transposes), drained by DVE copies; a bf16 cast of g_h1
    feeds the FC.
  - FC bias is added during the DVE PSUM drain (broadcast bias tile), and FC
    matmul chunks are interleaved into the tanh/transpose latency gaps of the
    NEXT step to keep the PE array busy.
"""
import sys

sys.path.insert(0, "/opt/trn_rl_repo")

from contextlib import ExitStack

import numpy as np
import ml_dtypes

import concourse.bass as bass
import concourse.tile as tile
from concourse import bacc, mybir
from concourse.bass_utils import run_bass_kernel_spmd

H = 1024
O = 8192
L = 2
T = 64
B = 256
N_CORES = 8
BG = B // 2          # batch rows per core (2-way DP)
OS = O // 4          # fc output slice per core (4-way TP)
KT = H // 128        # 8 k-tiles per 1024 contraction
F32 = mybir.dt.float32
F32R = mybir.dt.float32r
BF16 = mybir.dt.bfloat16

_cached = {}

RNN_W_NAMES = ["ih0", "hh0", "ih1", "hh1"]


def _build_program(n_steps: int, skip_fc=False, skip_tr=False, skip_rnn=False):
    nc = bacc.Bacc("TRN2", target_bir_lowering=False, debug=False, num_devices=N_CORES)

    # --- DRAM parameters (per-core shards, host-prepared layouts) ---
    # RNN weights, transposed+tiled on host: [p][k][n] of W.T, fp32
    wd = {}
    for nm in RNN_W_NAMES:
        wd[nm] = nc.declare_dram_parameter(f"w_{nm}", [128, KT, H], F32R,
                                           isOutput=False)
    w_fc = nc.declare_dram_parameter("w_fc", [128, KT, OS], BF16, isOutput=False)
    # initial state, g form: [p][k][b] = state[b, k*128+p], fp32
    gd = {}
    for nm in ("x", "h0", "h1"):
        gd[nm] = nc.declare_dram_parameter(f"g_{nm}", [128, KT, BG], F32R,
                                           isOutput=False)
    # bias bcast tiles (b_ih + b_hh per layer), fc bias bcast, identity
    bd = {}
    for l in range(L):
        bd[f"b{l}"] = nc.declare_dram_parameter(f"b{l}", [128, H], F32,
                                                isOutput=False)
    fcbd = nc.declare_dram_parameter("fcb", [128, OS], BF16, isOutput=False)
    identd = nc.declare_dram_parameter("ident", [128, 128], F32R, isOutput=False)

    out_d = nc.declare_dram_parameter("out", [n_steps, 128, OS], F32, isOutput=True)

    with tile.TileContext(nc) as tc, ExitStack() as ctx:
        wpool = ctx.enter_context(tc.tile_pool(name="w", bufs=1))
        cpool = ctx.enter_context(tc.tile_pool(name="c", bufs=1))
        gp = ctx.enter_context(tc.tile_pool(name="gp", bufs=3))
        gbp = ctx.enter_context(tc.tile_pool(name="gbp", bufs=2))
        hp = ctx.enter_context(tc.tile_pool(name="h", bufs=2))
        logp = ctx.enter_context(tc.tile_pool(name="log", bufs=3))
        rnn_ps = ctx.enter_context(tc.tile_pool(name="rnnps", bufs=1, space="PSUM"))
        tr_ps = ctx.enter_context(tc.tile_pool(name="trps", bufs=2, space="PSUM"))
        fc_ps = ctx.enter_context(tc.tile_pool(name="fcps", bufs=2, space="PSUM"))

        # --- preamble: load weights/constants ---
        w = {}
        for nm, dram in wd.items():
            t_ = wpool.tile([128, KT, H], F32R, tag=f"w{nm}", name=f"w{nm}")
            nc.sync.dma_start(t_[:], dram[:])
            w[nm] = t_
        wfc = wpool.tile([128, KT, OS], BF16, tag="wfc")
        nc.sync.dma_start(wfc[:], w_fc[:])

        bb = {}
        for nm, dram in bd.items():
            t_ = cpool.tile([128, H], F32, tag=nm, name=nm)
            nc.sync.dma_start(t_[:], dram[:])
            bb[nm] = t_
        fcb = cpool.tile([128, OS], BF16, tag="fcb")
        ident = cpool.tile([128, 128], F32R, tag="ident")
        for t_, d_ in [(fcb, fcbd), (ident, identd)]:
            nc.sync.dma_start(t_[:], d_[:])

        # --- initial state ---
        def g_init(nm):
            g = gp.tile([128, KT, BG], F32R, tag="g", name="g")
            nc.sync.dma_start(g[:], gd[nm][:])
            return g

        g_x = g_init("x")
        g_h0 = g_init("h0")
        g_h1 = g_init("h1")
        # bf16 copy of g_h1 for the FC (cast once on DVE)
        g_h1b = gbp.tile([128, KT, BG], BF16, tag="gb", name="gb")
        nc.vector.tensor_copy(g_h1b[:], g_h1[:])

        def rnn_layer(g_in, g_h, w_in, w_h, b):
            """tanh(in @ W_ihT + h @ W_hhT + b) -> h_sbuf [128(B), H] f32r.

            Bias is added in-place on the DVE (PSUM += bias bcast tile), off
            the PE array; tanh on the scalar engine."""
            ps = rnn_ps.tile([128, H], F32, tag="rnnps")
            for nck in range(2):
                nsl = bass.ts(nck, 512)
                for pi, (lhs, rhs) in enumerate(((g_in, w_in), (g_h, w_h))):
                    for k in range(KT):
                        first = pi == 0 and k == 0
                        last = pi == 1 and k == KT - 1
                        nc.tensor.matmul(ps[:, nsl], lhs[:, k, :], rhs[:, k, nsl],
                                         start=first, stop=last)
                nc.vector.tensor_add(ps[:, nsl], ps[:, nsl], b[:, nsl])
            h_sb = hp.tile([128, H], F32R, tag="h")
            nc.scalar.activation(h_sb[:], ps[:], mybir.ActivationFunctionType.Tanh)
            return h_sb

        def to_g(h_sb, want_bf16=False):
            """PE-transpose [B, H] -> g form [H(p), B] (fp32r transposes).

            4 transposed 128x128 tiles per PSUM bank; each bank drained by a
            wide DVE copy. Optionally also emit a bf16 copy for the FC."""
            g = gp.tile([128, KT, BG], F32R, tag="g", name="g")
            gb = None
            if want_bf16:
                gb = gbp.tile([128, KT, BG], BF16, tag="gb", name="gb")
            for grp in range(2):
                pt = tr_ps.tile([128, 512], F32R, tag="trps", name="pt")
                for j in range(4):
                    k = grp * 4 + j
                    nc.tensor.transpose(pt[:, bass.ts(j, 128)],
                                        h_sb[:, bass.ts(k, 128)], ident[:])
                gs = g[:, grp * 4:(grp + 1) * 4, :]
                nc.vector.tensor_copy(gs, pt[:])
                if want_bf16:
                    nc.vector.tensor_copy(gb[:, grp * 4:(grp + 1) * 4, :], pt[:])
            return g, gb

        def emit_fc_chunk(gb, tprev, ci):
            """FC chunk: logits[:, ci*512:(ci+1)*512] for step tprev (bf16)."""
            ps = fc_ps.tile([128, 512], F32, tag="fcps", name="fps")
            fsl = bass.ts(ci, 512)
            for k in range(KT):
                nc.tensor.matmul(ps[:], gb[:, k, :], wfc[:, k, fsl],
                                 start=(k == 0), stop=(k == KT - 1))
            lsb = logp.tile([128, 512], F32, tag="log", name="lsb")
            nc.vector.tensor_add(lsb[:], ps[:], fcb[:, fsl])
            nc.sync.dma_start(out_d[tprev][:, fsl], lsb[:])

        # Software pipeline: FC of step t-1 is interleaved into step t's
        # tanh/transpose gaps. pending = (g_h1_bf16, t_index) awaiting FC.
        pending = (g_h1b, None)  # g from init; no FC for it

        for t in range(n_steps):
            gb_prev, tprev = pending
            if not skip_rnn:
                h0_sb = rnn_layer(g_x, g_h0, w["ih0"], w["hh0"], bb["b0"])
                if not skip_fc and tprev is not None:
                    emit_fc_chunk(gb_prev, tprev, 0)
                if not skip_tr:
                    g_h0, _ = to_g(h0_sb)
                if not skip_fc and tprev is not None:
                    emit_fc_chunk(gb_prev, tprev, 1)
                h1_sb = rnn_layer(g_h0, g_h1, w["ih1"], w["hh1"], bb["b1"])
                if not skip_fc and tprev is not None:
                    emit_fc_chunk(gb_prev, tprev, 2)
                if not skip_tr:
                    g_h1, g_h1b = to_g(h1_sb, want_bf16=True)
                if not skip_fc and tprev is not None:
                    emit_fc_chunk(gb_prev, tprev, 3)
                g_x = g_h1
            pending = (g_h1b, t)

        # drain the last step's FC
        gb_prev, tprev = pending
        if not skip_fc and tprev is not None:
            for ci in range(4):
                emit_fc_chunk(gb_prev, tprev, ci)

    nc.finalize()
    return nc


def _prep_inputs(x, hidden, W_ih, W_hh, b_ih, b_hh, fc_W, fc_b, n_steps):
    """Build the 8 per-core input maps (host-side transposes)."""
    def gform(a):  # [BG, H] f32 -> [128, KT, BG]: out[p, k, b] = a[b, k*128+p]
        return np.ascontiguousarray(
            a.T.reshape(KT, 128, BG).transpose(1, 0, 2)).astype(np.float32)

    def wform(Wmat):  # [H_out, H_in] -> [128, KT, H_out] of W.T (f32)
        return np.ascontiguousarray(
            Wmat.T.reshape(KT, 128, Wmat.shape[0]).transpose(1, 0, 2)).astype(
                np.float32)

    ident = np.eye(128, dtype=np.float32)

    common = {"ident": ident}
    for l, nm_pair in enumerate([("ih0", "hh0"), ("ih1", "hh1")]):
        for nm, Wmat in zip(nm_pair, (W_ih[l], W_hh[l])):
            common[f"w_{nm}"] = wform(Wmat)
        common[f"b{l}"] = np.broadcast_to(
            (b_ih[l] + b_hh[l]).astype(np.float32).reshape(1, H),
            (128, H)).copy()

    in_maps = []
    for c in range(N_CORES):
        bg, j = c // 4, c % 4
        bsl = slice(bg * BG, (bg + 1) * BG)
        osl = slice(j * OS, (j + 1) * OS)
        wfc = np.ascontiguousarray(
            fc_W[osl].T.reshape(KT, 128, OS).transpose(1, 0, 2)).astype(
                ml_dtypes.bfloat16)
        m = dict(common)
        m["w_fc"] = wfc
        m["fcb"] = np.broadcast_to(
            fc_b[osl].astype(ml_dtypes.bfloat16).reshape(1, OS),
            (128, OS)).copy()
        for nm, src in (("x", x[0, bsl]), ("h0", hidden[0, bsl]),
                        ("h1", hidden[1, bsl])):
            m[f"g_{nm}"] = gform(src)
        in_maps.append(m)
    return in_maps


def kernel(x, hidden, embedded, W_ih, W_hh, b_ih, b_hh, fc_W, fc_b,
           _trace=False, _trace_kwargs=None):
    n_steps = embedded.shape[0]
    key = n_steps
    if key not in _cached:
        _cached[key] = _build_program(n_steps)
    nc = _cached[key]

    in_maps = _prep_inputs(np.asarray(x), np.asarray(hidden), np.asarray(W_ih),
                           np.asarray(W_hh), np.asarray(b_ih), np.asarray(b_hh),
                           np.asarray(fc_W), np.asarray(fc_b), n_steps)
    core_ids = list(range(N_CORES))
    res = run_bass_kernel_spmd(nc, in_maps, core_ids, trace=_trace,
                               **(_trace_kwargs or {}))

    out = np.empty((n_steps, 1, B, O), np.float32)
    for c in range(N_CORES):
        bg, j = c // 4, c % 4
        out[:, 0, bg * BG:(bg + 1) * BG, j * OS:(j + 1) * OS] = res.results[c]["out"]
    if _trace:
        kernel.last_results = res
    return out
